# revision 6
# baseline (speedup 1.0000x reference)
"""Trainium2 Bass kernel for nn_AELoss (MSE + smooth loss), 8-core data-parallel.

Strategy
--------
Shard batch dim (2048) across 8 cores -> 256 rows/core. Per core, 6 steps of
(b-group of 128, c); each step DMA-loads x,y tiles [128, t-chunk, 25] with
SWDGE f32->bf16 cast (HBM reads stay f32; all on-chip compute runs in bf16,
so DVE tensor_tensor hits its 2x perf mode).

Math: working in sum/difference space kills most of the work. With
d = x - y and p = x^2 - y^2:
    s_in - s_tgt per (b,c,j) = sum_t d - sum_t p + p[0] - d[T-1]
    total[b,c] = sum_{j<J-1} |s_in - s_tgt|;  smooth = mean sqrt(total)/(J*T)
    mse = mean d^2

Engine split (v5): DVE and ACT are load-balanced per t-quarter. On 'a'
quarters the Scalar engine squares x and y in place (qx = x^2, qy = y^2;
unary ACT work) and the DVE computes p = qx - qy in one pass; on 'd'
quarters the DVE does the classic butterfly (s = x+y, p = s*d). This moves
~2 of the DVE's 5 full passes onto the otherwise half-idle ACT engine.
The ACT-dependent p-subs plus the fold run one step late in the DVE queue
(software pipelining) so a busy ACT never stalls independent DVE work.
The fold halves in place on sd (300->150->75, boundary terms snapshotted
first) then finishes in a scratch tree -- fewer cycles than the one-shot
tree. ACT also squares d with accum_out for the per-partition MSE partial.
GpSimd only issues cast-DMAs -- real GpSimd compute poisons DVE via the
shared SBUF port. Per-core partials return as [128, 20]; host combines.
"""

import os
import sys

for _p in ("/opt/trn_rl_repo", "/root/.axon_site"):
    if os.path.isdir(_p) and _p not in sys.path:
        sys.path.insert(0, _p)

import numpy as np

# bass_utils imports antenv.axon_hooks when tracing is requested (e.g. via a
# BASS_TRACE env var); the module is missing in this image, so register a
# benign stub unless someone already provided a real one.
try:
    import antenv.axon_hooks  # noqa: F401
except ImportError:
    import types

    import antenv

    _hooks = types.ModuleType("antenv.axon_hooks")
    _hook_box = [None]
    _hooks.set_axon_ntff_profile_hook = lambda h: _hook_box.__setitem__(0, h)
    _hooks.get_axon_ntff_profile_hook = lambda: _hook_box[0]
    sys.modules["antenv.axon_hooks"] = _hooks
    antenv.axon_hooks = _hooks

import concourse.bass as bass
import concourse.tile as tile
from concourse import bacc, bass_isa, mybir
from concourse.bass_utils import run_bass_kernel_spmd

N_CORES = 8
B, C, T, J = 2048, 3, 300, 25
B_LOC = B // N_CORES          # 256 batch rows per core
P = 128                       # SBUF partitions
NG = B_LOC // P               # 2 b-groups per core
F32 = mybir.dt.float32
BF16 = mybir.dt.bfloat16
NSTEP = NG * C                # 6 (b-group, c) steps
Q = T // 4                    # 75-row compute quarters


def _fold(nc, fs_pool, src, res, tlen):
    """Sum src [P, 2, tlen, 25] over t -> res [P, 2, 25] f32 (additive).

    Halve in place on src while even (300->150->75), then finish with a
    binary tree in a scratch tile. Boundary values inside src[0:tlen/2]
    are clobbered -- snapshot them first.
    """
    v = nc.vector
    t = tlen
    while t % 2 == 0 and t > 80:
        h = t // 2
        v.tensor_add(src[:, :, 0:h, :], src[:, :, 0:h, :], src[:, :, h:t, :])
        t = h
    n0 = 1 << (t.bit_length() - 2)
    rest = t - 2 * n0
    fs = fs_pool.tile([P, 2, 128, J], BF16, tag="fold_bf")
    v.tensor_add(fs[:, :, 0:n0, :], src[:, :, 0:n0, :], src[:, :, n0 : 2 * n0, :])
    if rest:
        v.tensor_add(
            fs[:, :, 0:rest, :], fs[:, :, 0:rest, :], src[:, :, 2 * n0 : t, :]
        )
    n = n0 // 2
    while n >= 2:
        v.tensor_add(fs[:, :, 0:n, :], fs[:, :, 0:n, :], fs[:, :, n : 2 * n, :])
        n //= 2
    v.tensor_add(res[:, :, :], fs[:, :, 0, :], fs[:, :, 1, :])


def _body(tc, nc, x_d, y_d, out_d):
    cfg = CFG

    with (
        tc.tile_pool(name="inp", bufs=cfg["xy"]) as inp_pool,
        tc.tile_pool(name="sd", bufs=cfg["sd"]) as sd_pool,
        tc.tile_pool(name="fold", bufs=cfg["fold"]) as fold_pool,
        tc.tile_pool(name="small", bufs=4) as small_pool,
        tc.tile_pool(name="junk", bufs=1) as junk_pool,
        tc.tile_pool(name="persist", bufs=1) as persist,
    ):
        totals6 = persist.tile([P, NSTEP], F32)       # per-step sum_j |s_in - s_tgt|
        nch0, nchm = cfg.get("nch0", 4), cfg.get("nchm", 2)
        nchunk = nch0 + (NSTEP - 1) * nchm
        mse14 = persist.tile([P, nchunk], F32)        # per-chunk sum (x-y)^2
        junk = junk_pool.tile([P, T // nchm, J], BF16)

        def emit_finish(kk, sd, yts, paths):
            """ACT-dependent p-subs, boundary snapshot, fold, |D| reduce."""
            last = kk == NSTEP - 1

            def yslice(q0, q1):
                for yt, t0, t1 in yts:
                    if t0 <= q0 and q1 <= t1:
                        return yt[:, q0 - t0 : q1 - t0, :]
                raise AssertionError("quarter spans chunks")

            # p = qx - qy for the ACT quarters
            for q, path in enumerate(paths):
                q0, q1 = q * Q, (q + 1) * Q
                if path == "a":
                    nc.vector.tensor_sub(
                        sd[:, 0, q0:q1, :], sd[:, 0, q0:q1, :], yslice(q0, q1)
                    )
            # snapshot p[0] - d[T-1] before the in-place fold clobbers t=0
            bnd = small_pool.tile([P, J], F32, tag="bnd")
            nc.vector.tensor_sub(bnd[:, :], sd[:, 0, 0, :], sd[:, 1, T - 1, :])

            res = small_pool.tile([P, 2, J], F32, tag="res")
            if last and cfg.get("tailfold", True):
                # fold per t-half so the first half's chain overlaps the
                # second half's DMA -> shorter tail
                ra = small_pool.tile([P, 2, J], F32, tag="res_a")
                _fold(nc, fold_pool, sd[:, :, 0:150, :], ra, tlen=150)
                rb = small_pool.tile([P, 2, J], F32, tag="res_b")
                _fold(nc, fold_pool, sd[:, :, 150:300, :], rb, tlen=150)
                nc.vector.tensor_add(res[:, :, :], ra[:, :, :], rb[:, :, :])
            else:
                _fold(nc, fold_pool, sd, res, tlen=T)

            # D[j] = (Ad - Pd) + (p[0] - d[T-1])
            D = small_pool.tile([P, J], F32, tag="D")
            nc.vector.tensor_sub(D[:, :], res[:, 1, :], res[:, 0, :])
            nc.vector.tensor_add(D[:, :], D[:, :], bnd[:, :])
            nc.vector.reduce_sum(
                totals6[:, kk : kk + 1],
                D[:, 0 : J - 1],
                axis=mybir.AxisListType.X,
                apply_absolute_value=True,
            )

        pending = None
        k = 0
        mcol = 0
        for g in range(NG):
            for c in range(C):
                last = k == NSTEP - 1
                # x is DMA'd straight into sd[:,0] (-> qx or s -> p);
                # d = x - y goes to sd[:,1]
                sd = sd_pool.tile([P, 2, T, J], BF16, tag="sd")
                paths = cfg["path_last"] if last else cfg["path"]

                # DMA chunks: fine for step 0 so compute starts sooner
                nch = nch0 if k == 0 else nchm
                tc_sz = T // nch
                yts = []
                for h in range(nch):
                    t0, t1 = h * tc_sz, (h + 1) * tc_sz
                    nc.gpsimd.dma_start(
                        out=sd[:, 0, t0:t1, :],
                        in_=x_d[g * P : (g + 1) * P, c, t0:t1, :],
                    )
                    yt = inp_pool.tile([P, tc_sz, J], BF16, tag="y")
                    nc.gpsimd.dma_start(
                        out=yt[:, :, :],
                        in_=y_d[g * P : (g + 1) * P, c, t0:t1, :],
                    )
                    yts.append((yt, t0, t1))

                    # d = x - y for this chunk (reads x before it's squared)
                    nc.vector.tensor_sub(sd[:, 1, t0:t1, :], sd[:, 0, t0:t1, :], yt)

                    # MSE partial: sum d^2 on ACT (junk elementwise output
                    # to the scratch tile; y must survive for qy)
                    nc.scalar.activation(
                        junk[:, 0 : t1 - t0, :],
                        sd[:, 1, t0:t1, :],
                        mybir.ActivationFunctionType.Square,
                        accum_out=mse14[:, mcol : mcol + 1],
                    )
                    mcol += 1

                def yslice(q0, q1):
                    for yt, t0, t1 in yts:
                        if t0 <= q0 and q1 <= t1:
                            return yt[:, q0 - t0 : q1 - t0, :]
                    raise AssertionError("quarter spans chunks")

                # 'a' quarters: ACT squares x,y in place (DVE does p = qx-qy
                # one step later); 'd' quarters: DVE butterfly now
                for q, path in enumerate(paths):
                    q0, q1 = q * Q, (q + 1) * Q
                    if path == "a":
                        nc.scalar.activation(
                            sd[:, 0, q0:q1, :],
                            sd[:, 0, q0:q1, :],
                            mybir.ActivationFunctionType.Square,
                        )
                        nc.scalar.activation(
                            yslice(q0, q1),
                            yslice(q0, q1),
                            mybir.ActivationFunctionType.Square,
                        )
                    else:
                        nc.vector.tensor_add(
                            sd[:, 0, q0:q1, :], sd[:, 0, q0:q1, :], yslice(q0, q1)
                        )
                        nc.vector.tensor_mul(
                            sd[:, 0, q0:q1, :], sd[:, 0, q0:q1, :], sd[:, 1, q0:q1, :]
                        )

                # software pipeline: finish the PREVIOUS step now (its ACT
                # squares are long done -> DVE never waits), except the last
                # step which finishes immediately to keep the tail short.
                if pending is not None:
                    emit_finish(*pending)
                pending = (k, sd, yts, paths)
                if last:
                    emit_finish(*pending)
                    pending = None

                k += 1

        # tail: ship the raw per-partition partials; sqrt + final sums happen
        # on the host. Issue the early-ready pieces first -- the Sync queue
        # is in-order, so only the last step's 512B totals column rides the
        # critical path.
        nc.sync.dma_start(out=out_d[:, NSTEP:], in_=mse14[:, :])
        nc.sync.dma_start(
            out=out_d[:, 0 : NSTEP - 1], in_=totals6[:, 0 : NSTEP - 1]
        )
        nc.sync.dma_start(
            out=out_d[:, NSTEP - 1 : NSTEP], in_=totals6[:, NSTEP - 1 : NSTEP]
        )


_NC_CACHE = None
CFG = {
    "xy": 6,
    "sd": 4,
    "fold": 2,
    "nch0": 4,
    "nchm": 2,
    "tailfold": True,
    "path": "aaad",
    "path_last": "aadd",
}


def _build():
    global _NC_CACHE
    if _NC_CACHE is not None:
        return _NC_CACHE
    nc = bacc.Bacc("TRN2", target_bir_lowering=False, debug=False, num_devices=N_CORES)
    x_d = nc.dram_tensor("inputs", [B_LOC, C, T, J], F32, kind="ExternalInput")
    y_d = nc.dram_tensor("targets", [B_LOC, C, T, J], F32, kind="ExternalInput")
    nchunk = CFG.get("nch0", 4) + (NSTEP - 1) * CFG.get("nchm", 2)
    out_d = nc.dram_tensor("out", [P, NSTEP + nchunk], F32, kind="ExternalOutput")
    with tile.TileContext(nc) as tc:
        _body(tc, nc, x_d.ap(), y_d.ap(), out_d.ap())
    nc.compile()
    _NC_CACHE = nc
    return nc


def _run(inputs, targets, trace=False, **kw):
    nc = _build()
    inputs = np.ascontiguousarray(inputs, dtype=np.float32)
    targets = np.ascontiguousarray(targets, dtype=np.float32)
    in_maps = [
        {
            "inputs": inputs[i * B_LOC : (i + 1) * B_LOC],
            "targets": targets[i * B_LOC : (i + 1) * B_LOC],
        }
        for i in range(N_CORES)
    ]
    res = run_bass_kernel_spmd(
        nc, in_maps, core_ids=list(range(N_CORES)), trace=trace, **kw
    )
    mse_sum = 0.0
    smooth_sum = 0.0
    for i in range(N_CORES):
        o = np.asarray(res.results[i]["out"], dtype=np.float64)  # [P, 6+nchunk]
        totals = o[:, :NSTEP]
        smooth_sum += float(np.sqrt(totals).sum()) / (J * T)
        mse_sum += float(o[:, NSTEP:].sum())
    value = 2.0 * (mse_sum / (B * C * T * J)) + 3.0 * (smooth_sum / (B * C))
    return np.array(value, dtype=np.float32), res


def kernel(inputs, targets):
    value, _ = _run(inputs, targets)
    return value
